# revision 56
# baseline (speedup 1.0000x reference)
# Trainium2 Bass kernel for nn_Attention_60464549593105.
#
# Math (per batch b, spatial point (h,w), seq s):
#   energy[k] = tanh( We @ enc[:,s] + Wh @ hidden + b_att )      (K=128)
#   score[s]  = W_v . energy
#   out[s]    = softmax_s(score)
#
# Strategy: shard the H axis across 8 cores (8 rows each) so softmax over
# seq is core-local (no collectives).
#
# Changes vs the 192us baseline (which was ACT(tanh)-bound at FD=1024
# psum tiles, 1147ns per 2-seq chunk, with DVE adds at 1224ns/chunk);
# measured 169.5us at nominal clock (chip throttling adds up to +20%
# run-to-run; normalize comparisons by the FD=3072 tanh dur, 2854ns
# nominal):
#   - steady cycle = [E(2,2,2), P(2)] per 8 seq. "E" chunks run the
#     proj_h add on DVE writing bf16 into a big SBUF staging tile; one
#     tanh per group at FD=3072 from SBUF cuts the 352-cycle/instr ACT
#     overhead from 34% to ~11% (573 -> 476 ns/seq). "P" chunks keep the
#     add on the PE (accumulating whT matmul, 216ns/seq vs DVE's 611) with
#     tanh from PSUM at FD=1024, keeping the 1x-rate DVE under the ACT
#     wall. ACT ~510ns/seq is the steady-state pacer; DVE ~490, PE ~450.
#   - scores use 4-way column-tiled matmuls: seq s -> col group s%4, row
#     s//4, all 64 scores in ONE psum bank [128,512]; the 4 col groups'
#     matmuls run concurrently in the PE array (tile_position explicit —
#     the auto-derive path rejects base partition 96).
#   - score matmuls are deferred >=2 chunks after their tanh (cross-batch
#     carry-over list) so the PE's in-order queue never head-blocks on a
#     just-issued tanh; this was worth ~10us.
#   - softmax tail (exp included) is deferred into the next batch at
#     gi==2: the batch boundary costs ACT nothing. Sum via masked
#     ones-matmul broadcast to 128 rows (mask zeroes the 64 unused score
#     rows); recip on DVE, mul on gpsimd; out DMA'd per col group (out
#     dram layout [B, 16, 4, FREE] == s-order). The last batch's sums go
#     to a free epsum tile (bank per split, no serialization) and its out
#     DMAs spread across sync/gpsimd/scalar queues to shorten the drain.
#   - ramp: packed weight DMA split so weT/whT don't wait for wvs/zeros,
#     batt on the scalar queue, enc chunk DMAs spread over gpsimd+scalar
#     queues with 1/1/2/4/8/12... seq sizing (first DMA on a queue takes
#     ~4us to land; enc prefetch was the old ramp bottleneck), wvs/zeros +
#     hid[1:] DMAs deferred off the ramp, proj_h for batch b+1 emitted
#     mid-batch b. First tanh ~15us (framework preamble is ~7us).
# Things measured and rejected: gpsimd partition_all_reduce for the
# softmax sum (blocks the gpsimd queue ~5us, +19us); moving the deferred
# tail to gi==3 (+5us); P-heavier batch-0 ramp (+5us); staging the
# mid-batch sums in an epsum tile (steals an e_ps ring slot from the
# chunk pipeline for the whole tail, +10us).

import numpy as np
import ml_dtypes

B, D, E, S, H, W = 4, 128, 128, 64, 64, 64
K = 128
NCORES = 8
HSH = H // NCORES          # h rows per core
FREE = HSH * W             # free-dim elements per (b, s) tile
SCH = 24                   # max seq positions per enc DMA chunk (3 MiB bf16)

_CACHE = {}


def _plan(b):
    # per-batch compute plan: list of groups.
    #   ('P', [n])        pe-add chunk, tanh from psum, n seq (1 or 2)
    #   ('E', [n1,n2..])  DVE-add chunks staged to SBUF, one tanh over sum
    cyc = [('E', [2, 2, 2]), ('P', [2])]          # 8 seq steady cycle
    if b == 0:
        # ramp: tiny pe-add chunks first (only need weT/whT/enc, not the
        # DVE proj_h copy), growing groups after.
        g = [('P', [1]), ('P', [1]), ('P', [2]), ('E', [2, 2])]
        g += cyc * 7
        return g
    if b < B - 1:
        return cyc * 8
    # last batch: steady start, tapering tail for a short drain
    g = cyc * 6
    g += [('E', [2, 2, 2]), ('P', [2]), ('E', [2, 2]), ('P', [2]),
          ('P', [1]), ('P', [1])]
    return g


def _dma_plan(b):
    # (chunk_seq, queue): 'g' = gpsimd queue, 'a' = scalar queue. Batch 0's
    # ramp spreads issues over two queues and uses mid-size chunks so the
    # enc prefetch isn't serialized behind one queue's ~0.65us issue slots.
    if b == 0:
        return [(1, 'g'), (1, 'a'), (2, 'g'), (4, 'a'), (8, 'g'),
                (12, 'g'), (12, 'g'), (12, 'g'), (12, 'g')]
    return [(24, 'g'), (24, 'g'), (16, 'g')]


def _build_bass():
    import concourse.bacc as bacc
    import concourse.mybir as mybir
    import concourse.tile as tile
    from contextlib import ExitStack

    f32 = mybir.dt.float32
    f32r = mybir.dt.float32r
    bf16 = mybir.dt.bfloat16
    AF = mybir.ActivationFunctionType

    nc = bacc.Bacc("TRN2", target_bir_lowering=False, debug=False)
    enc = nc.dram_tensor("enc", [B, E, S * FREE], bf16, kind="ExternalInput")
    hidp = nc.dram_tensor("hidp", [D, B * FREE], bf16, kind="ExternalInput")
    # packed: cols 0:128 weT, 128:256 whT, 256:768 hid batch-0 slice (one
    # early DMA delivers everything the ramp's pe-add chunks need), 768:832
    # wvs (wv at col 799 = window col 31), 832:960 zeros (opener weights)
    wpack = nc.dram_tensor("wpack", [E, 960], bf16, kind="ExternalInput")
    batt = nc.dram_tensor("batt", [K, 1], f32, kind="ExternalInput")
    # out[b, r, g, :] holds seq s = 4r + g -> flattens to [B, S, FREE]
    out = nc.dram_tensor("out", [B, S // 4, 4, FREE], f32, kind="ExternalOutput")

    with tile.TileContext(nc) as tc, ExitStack() as ctx:
        consts = ctx.enter_context(tc.tile_pool(name="consts", bufs=1))
        encp = ctx.enter_context(tc.tile_pool(name="encp", bufs=3))
        epsum = ctx.enter_context(tc.tile_pool(name="epsum", bufs=3, space="PSUM"))
        # one score bank per batch, 2 alternating so batch b+1's scores
        # start while batch b's softmax tail drains
        scpsum = ctx.enter_context(tc.tile_pool(name="scpsum", bufs=2, space="PSUM"))
        ebp = ctx.enter_context(tc.tile_pool(name="ebp", bufs=5))
        thp = ctx.enter_context(tc.tile_pool(name="thp", bufs=5))
        smax = ctx.enter_context(tc.tile_pool(name="smax", bufs=3))

        # ---- consts (enc chunk 0 is issued by the batch loop on the
        # gpsimd queue and overlaps this single early sync-queue load;
        # everything the ramp's pe-add chunks need arrives in ONE DMA so
        # the first tanh isn't gated on a second ~2us DMA-pipe landing)
        wmain_sb = consts.tile([E, 768], bf16)
        nc.sync.dma_start(out=wmain_sb, in_=wpack[:, 0:768])
        weT_sb = wmain_sb[:, 0:128]
        whT_sb = wmain_sb[:, 128:256]
        hid0_v = wmain_sb[:, 256:768]
        # batt on the scalar queue: lands before the first tanh needs it,
        # without joining the sync queue's ramp DMA herd
        batt_sb = consts.tile([K, 1], f32)
        nc.scalar.dma_start(out=batt_sb, in_=batt[:])
        hid_sb = consts.tile([D, B * FREE], bf16)
        waux_sb = consts.tile([E, 192], bf16)
        wvs_sb = waux_sb[:, 0:64]
        zeros_sb = waux_sb[:, 64:192]
        late_state = {"done": False}

        def hid_slice(b):
            return hid0_v if b == 0 else hid_sb[:, b * FREE : (b + 1) * FREE]

        def emit_late_consts():
            # deferred off the ramp critical path (first needed: batch 0's
            # score flush / batch 1's pe-adds)
            if late_state["done"]:
                return
            late_state["done"] = True
            nc.sync.dma_start(out=waux_sb, in_=wpack[:, 768:960])
            nc.sync.dma_start(out=hid_sb[:, FREE:], in_=hidp[:, FREE:])

        # sum-matmul mask: 1.0 at the 64 valid score rows (32g+q, q<16).
        # Emitted lazily (after batch 0's ramp) so the DVE memsets don't
        # delay the first energy adds; first use is batch 0's softmax tail
        # early in batch 1.
        msk_tmp = consts.tile([128, 128], f32)
        mask_sb = consts.tile([128, 128], f32r)
        mask_state = {"done": False}

        def emit_mask():
            if mask_state["done"]:
                return
            mask_state["done"] = True
            nc.vector.memset(msk_tmp, 0.0)
            for g in range(4):
                nc.vector.memset(msk_tmp[32 * g : 32 * g + 16, :], 1.0)
            nc.vector.tensor_copy(mask_sb, msk_tmp)

        # proj_h (repeated 2x along free) per batch, f32 in SBUF
        projh2_sb = consts.tile([K, B, 2 * FREE], f32)

        def emit_projh(b):
            ph_ps = epsum.tile([K, 2 * FREE], f32, tag="e_ps", name="ph_ps")
            for jj in range(2):
                nc.tensor.matmul(ph_ps[:, jj * FREE : (jj + 1) * FREE],
                                 lhsT=whT_sb, rhs=hid_slice(b),
                                 start=True, stop=True)
            nc.vector.tensor_copy(projh2_sb[:, b, :], ph_ps)

        def softmax_tail(b, sc, expv, split=1, do_exp=True):
            # exp + sum over the 64 valid rows via masked ones-matmul,
            # broadcast to all 128 rows. The sums go to a free epsum tile
            # (one PSUM bank per split) so sum[p+1] doesn't serialize
            # behind recip[p] reading the same bank. exp lives here (off
            # the batch-boundary critical path).
            rec = smax.tile([128, FREE], f32, tag="rec", name="rec")
            ob = smax.tile([128, FREE], f32, tag="ob", name="ob")
            fs = FREE // split
            last = b == B - 1
            # last batch: sums go to a free epsum tile (one bank per split,
            # no serialization; the chunk pipeline is done so the ring is
            # idle). Mid-batch tails overwrite the consumed score bank so
            # they don't steal an e_ps ring slot from the chunk pipeline.
            if last:
                su = epsum.tile([K, 2 * FREE], f32, tag="e_ps", name="su")
                sslot = lambda p: su[:, p * FREE : p * FREE + fs]
            else:
                sslot = lambda p: sc[:, p * fs : p * fs + fs]
            # out DMAs spread across queues so their ~0.6us issue slots
            # don't serialize (matters for the final batch's drain)
            queues = ([nc.sync, nc.gpsimd, nc.scalar, nc.sync] if last
                      else [nc.sync, nc.sync, nc.gpsimd, nc.gpsimd])
            if do_exp:
                for p in range(split):
                    sl = slice(p * fs, (p + 1) * fs)
                    nc.scalar.activation(expv[:, sl], sc[:, sl], AF.Exp)
            for p in range(split):
                sl = slice(p * fs, (p + 1) * fs)
                nc.tensor.matmul(sslot(p), lhsT=mask_sb, rhs=expv[:, sl],
                                 start=True, stop=True, skip_group_check=True)
                nc.vector.reciprocal_approx_fast(out=rec[:, sl],
                                                 in_=sslot(p))
                # mul on the (otherwise idle) gpsimd engine keeps the
                # in-order DVE queue free for the energy adds; the last
                # batch splits muls across gpsimd+vector (both idle then)
                mule = nc.vector if (last and p == 1) else nc.gpsimd
                mule.tensor_mul(out=ob[:, sl], in0=expv[:, sl],
                                in1=rec[:, sl])
                for g in range(4):
                    queues[g].dma_start(
                        out=out[b, :, g, sl],
                        in_=ob[32 * g : 32 * g + 16, sl])

        pending_tail = None
        pending_scores = []   # (th, sbase, nseq, gci_created, sc_tile)
        gstate = {"ci": 0}    # global chunk counter (score-flush aging)

        def flush_scores(min_age=0, upto_batch=None):
            # emit score matmuls whose tanh was issued >= min_age chunks
            # ago: keeps the PE's in-order queue from stalling on a
            # just-issued tanh. Entries carry their own score bank, so
            # flushing crosses batch boundaries (no boundary burst).
            while pending_scores:
                th_p, sbase, nseq, cic, sc_t, eb_t = pending_scores[0]
                if upto_batch is not None:
                    if eb_t > upto_batch:
                        break
                elif gstate["ci"] - cic < min_age:
                    break
                pending_scores.pop(0)
                for jj in range(nseq):
                    s = sbase + jj
                    r, g = s // 4, s % 4
                    nc.tensor.matmul(
                        sc_t[32 * g : 32 * g + 32, :],
                        lhsT=wvs_sb[:, 31 - r : 63 - r],
                        rhs=th_p[:, jj * FREE : (jj + 1) * FREE],
                        start=False, stop=(s == S - 1),
                        skip_group_check=True,
                        tile_position=(0, 32 * g))

        for b in range(B):
            plan = _plan(b)
            dmas = _dma_plan(b)
            # score bank: opener matmul writes zeros to all 128 rows with
            # start=True (start clears has_written for the whole bank, so
            # only one matmul may use it); scores accumulate with
            # start=False. seq s -> col group s%4 row s//4, so consecutive
            # scores hit different PE col groups and run concurrently.
            sc = scpsum.tile([128, FREE], f32, tag="sc", name="sc")

            def emit_opener(sc=sc):
                nc.tensor.matmul(sc, lhsT=zeros_sb, rhs=hid0_v,
                                 start=True, stop=False,
                                 skip_group_check=True)

            if b > 0:
                emit_opener()
            expv = smax.tile([128, FREE], f32r, tag="expv", name="expv")

            # iterators over enc DMA tiles
            dma_iter = iter(dmas)
            dma_left = 0
            et = None
            et_off = 0     # first seq index held by current et tile
            s0 = 0         # next seq index to compute
            gi = 0         # group counter (for ramp bookkeeping)

            def next_dma_tile():
                nonlocal dma_left, et, et_off
                csz, q = next(dma_iter)
                et = encp.tile([E, SCH * FREE], bf16, tag="et", name="et")
                (nc.scalar if q == 'a' else nc.gpsimd).dma_start(
                    out=et[:, : csz * FREE],
                    in_=enc[b, :, s0 * FREE : (s0 + csz) * FREE])
                et_off = s0
                dma_left = csz

            for mode, chunks in plan:
                nseq = sum(chunks)
                if mode == 'E':
                    eb = ebp.tile([K, 3 * 2 * FREE], bf16, tag="eb", name="eb")
                    th = thp.tile([K, 3 * 2 * FREE], bf16, tag="the", name="the")
                else:
                    th = thp.tile([K, 2 * FREE], bf16, tag="thp", name="thp")
                off = 0
                for csz in chunks:
                    if dma_left < csz:
                        assert dma_left == 0, (b, mode, chunks, dma_left)
                        next_dma_tile()
                    j0 = s0 - et_off
                    e_ps = epsum.tile([K, 2 * FREE], f32, tag="e_ps",
                                      name="e_ps")
                    for jj in range(csz):
                        nc.tensor.matmul(
                            e_ps[:, jj * FREE : (jj + 1) * FREE],
                            lhsT=weT_sb,
                            rhs=et[:, (j0 + jj) * FREE : (j0 + jj + 1) * FREE],
                            start=True, stop=(mode == 'E'))
                    if mode == 'P':
                        for jj in range(csz):
                            nc.tensor.matmul(
                                e_ps[:, jj * FREE : (jj + 1) * FREE],
                                lhsT=whT_sb, rhs=hid_slice(b),
                                start=False, stop=True)
                    flush_scores(min_age=1 if (b == B - 1 and s0 >= 52)
                                 else 2)
                    if mode == 'E':
                        nc.vector.tensor_add(
                            out=eb[:, off * FREE : (off + csz) * FREE],
                            in0=e_ps[:, : csz * FREE],
                            in1=projh2_sb[:, b, : csz * FREE])
                    else:
                        nc.scalar.activation(th[:, : csz * FREE],
                                             e_ps[:, : csz * FREE],
                                             AF.Tanh, bias=batt_sb)
                        pending_scores.append((th, s0, csz, gstate["ci"],
                                               sc, b))
                    off += csz
                    s0 += csz
                    dma_left -= csz
                    gstate["ci"] += 1
                if mode == 'E':
                    nc.scalar.activation(th[:, : nseq * FREE],
                                         eb[:, : nseq * FREE],
                                         AF.Tanh, bias=batt_sb)
                    pending_scores.append((th, s0 - nseq, nseq,
                                           gstate["ci"] - 1, sc, b))
                gi += 1
                if b == 0 and gi == 1:
                    emit_late_consts()
                    emit_opener()
                    emit_projh(0)
                if b == 0 and gi == 4:
                    emit_mask()
                if gi == 6 and b + 1 < B:
                    # next batch's proj_h mid-batch: its PE matmuls and DVE
                    # copy land where there's slack, not at the boundary
                    emit_projh(b + 1)
                if gi == 2 and pending_tail is not None:
                    flush_scores(upto_batch=pending_tail[0])
                    softmax_tail(*pending_tail)
                    pending_tail = None
            if b < B - 1:
                pending_tail = (b, sc, expv)
            else:
                flush_scores()
                softmax_tail(b, sc, expv, split=2)
    nc.compile()
    return nc


def _get_bass():
    if "nc" not in _CACHE:
        _CACHE["nc"] = _build_bass()
    return _CACHE["nc"]


def kernel(hidden_state, encoder_outputs, W_att, b_att, W_v):
    from concourse.bass_utils import run_bass_kernel_spmd

    bf16 = ml_dtypes.bfloat16
    hidden_state = np.asarray(hidden_state, dtype=np.float32)
    W_att = np.asarray(W_att, dtype=np.float32)
    b_att = np.asarray(b_att, dtype=np.float32)
    W_v = np.asarray(W_v, dtype=np.float32)
    enc_bf = np.asarray(encoder_outputs, dtype=np.float32).astype(bf16)

    wpack0 = np.zeros((E, 960), dtype=np.float32)
    wpack0[:, 0:128] = W_att[:, D:].T         # weT
    wpack0[:, 128:256] = W_att[:, :D].T       # whT
    wpack0[:, 768 + 31] = W_v[0]              # wvs window, wv at col 31
    wpack0 = wpack0.astype(bf16)
    batt = np.ascontiguousarray(b_att.reshape(K, 1))

    in_maps = []
    for c in range(NCORES):
        h0 = c * HSH
        enc_c = np.ascontiguousarray(
            enc_bf[:, :, :, h0 : h0 + HSH, :]
        ).reshape(B, E, S * FREE)
        hid_c = np.ascontiguousarray(
            hidden_state[:, :, h0 : h0 + HSH, :].transpose(1, 0, 2, 3)
        ).reshape(D, B * FREE).astype(bf16)
        wpack = wpack0.copy()
        wpack[:, 256:768] = hid_c[:, :FREE]   # hid batch-0 slice
        in_maps.append(
            {"enc": enc_c, "hidp": hid_c, "wpack": wpack, "batt": batt}
        )

    nc = _get_bass()
    kwargs = dict(_CACHE.get("run_kwargs", {}))
    res = run_bass_kernel_spmd(nc, in_maps, core_ids=list(range(NCORES)), **kwargs)
    _CACHE["last_result"] = res
    shards = [r["out"].reshape(B, S, HSH, W) for r in res.results]
    return np.concatenate(shards, axis=2)


# revision 58
# speedup vs baseline: 1.0050x; 1.0050x over previous
# Trainium2 Bass kernel for nn_Attention_60464549593105.
#
# Math (per batch b, spatial point (h,w), seq s):
#   energy[k] = tanh( We @ enc[:,s] + Wh @ hidden + b_att )      (K=128)
#   score[s]  = W_v . energy
#   out[s]    = softmax_s(score)
#
# Strategy: shard the H axis across 8 cores (8 rows each) so softmax over
# seq is core-local (no collectives).
#
# Changes vs the 192us baseline (which was ACT(tanh)-bound at FD=1024
# psum tiles, 1147ns per 2-seq chunk, with DVE adds at 1224ns/chunk);
# measured 169.5us at nominal clock (chip throttling adds up to +20%
# run-to-run; normalize comparisons by the FD=3072 tanh dur, 2854ns
# nominal):
#   - steady cycle = [E(2,2,2), P(2)] per 8 seq. "E" chunks run the
#     proj_h add on DVE writing bf16 into a big SBUF staging tile; one
#     tanh per group at FD=3072 from SBUF cuts the 352-cycle/instr ACT
#     overhead from 34% to ~11% (573 -> 476 ns/seq). "P" chunks keep the
#     add on the PE (accumulating whT matmul, 216ns/seq vs DVE's 611) with
#     tanh from PSUM at FD=1024, keeping the 1x-rate DVE under the ACT
#     wall. ACT ~510ns/seq is the steady-state pacer; DVE ~490, PE ~450.
#   - scores use 4-way column-tiled matmuls: seq s -> col group s%4, row
#     s//4, all 64 scores in ONE psum bank [128,512]; the 4 col groups'
#     matmuls run concurrently in the PE array (tile_position explicit —
#     the auto-derive path rejects base partition 96).
#   - score matmuls are deferred >=2 chunks after their tanh (cross-batch
#     carry-over list) so the PE's in-order queue never head-blocks on a
#     just-issued tanh; this was worth ~10us.
#   - softmax tail (exp included) is deferred into the next batch at
#     gi==2: the batch boundary costs ACT nothing. Sum via masked
#     ones-matmul broadcast to 128 rows (mask zeroes the 64 unused score
#     rows); recip on DVE, mul on gpsimd; out DMA'd per col group (out
#     dram layout [B, 16, 4, FREE] == s-order). The last batch's sums go
#     to a free epsum tile (bank per split, no serialization) and its out
#     DMAs spread across sync/gpsimd/scalar queues to shorten the drain.
#   - ramp: packed weight DMA split so weT/whT don't wait for wvs/zeros,
#     batt on the scalar queue, enc chunk DMAs spread over gpsimd+scalar
#     queues with 1/1/2/4/8/12... seq sizing (first DMA on a queue takes
#     ~4us to land; enc prefetch was the old ramp bottleneck), wvs/zeros +
#     hid[1:] DMAs deferred off the ramp, proj_h for batch b+1 emitted
#     mid-batch b. First tanh ~15us (framework preamble is ~7us).
# Things measured and rejected: gpsimd partition_all_reduce for the
# softmax sum (blocks the gpsimd queue ~5us, +19us); moving the deferred
# tail to gi==3 (+5us); P-heavier batch-0 ramp (+5us); staging the
# mid-batch sums in an epsum tile (steals an e_ps ring slot from the
# chunk pipeline for the whole tail, +10us).

import numpy as np
import ml_dtypes

B, D, E, S, H, W = 4, 128, 128, 64, 64, 64
K = 128
NCORES = 8
HSH = H // NCORES          # h rows per core
FREE = HSH * W             # free-dim elements per (b, s) tile
SCH = 24                   # max seq positions per enc DMA chunk (3 MiB bf16)

_CACHE = {}


def _plan(b):
    # per-batch compute plan: list of groups.
    #   ('P', [n])        pe-add chunk, tanh from psum, n seq (1 or 2)
    #   ('E', [n1,n2..])  DVE-add chunks staged to SBUF, one tanh over sum
    cyc = [('E', [2, 2, 2]), ('P', [2])]          # 8 seq steady cycle
    if b == 0:
        # ramp: tiny pe-add chunks first (only need weT/whT/enc, not the
        # DVE proj_h copy), growing groups after.
        g = [('P', [1]), ('P', [1]), ('P', [2]), ('E', [2, 2])]
        g += cyc * 7
        return g
    if b < B - 1:
        return cyc * 8
    # last batch: steady start, tapering tail for a short drain
    g = cyc * 6
    g += [('E', [2, 2, 2]), ('P', [2]), ('E', [2, 2]), ('P', [2]),
          ('P', [1]), ('P', [1])]
    return g


def _dma_plan(b):
    # (chunk_seq, queue): 'g' = gpsimd queue, 'a' = scalar queue. Batch 0's
    # ramp spreads issues over two queues and uses mid-size chunks so the
    # enc prefetch isn't serialized behind one queue's ~0.65us issue slots.
    if b == 0:
        return [(1, 'g'), (1, 'a'), (2, 'g'), (4, 'a'), (8, 'g'),
                (12, 'g'), (12, 'g'), (12, 'g'), (12, 'g')]
    return [(24, 'g'), (24, 'g'), (16, 'g')]


def _build_bass():
    import concourse.bacc as bacc
    import concourse.mybir as mybir
    import concourse.tile as tile
    from contextlib import ExitStack

    f32 = mybir.dt.float32
    f32r = mybir.dt.float32r
    bf16 = mybir.dt.bfloat16
    AF = mybir.ActivationFunctionType

    nc = bacc.Bacc("TRN2", target_bir_lowering=False, debug=False)
    enc = nc.dram_tensor("enc", [B, E, S * FREE], bf16, kind="ExternalInput")
    hidp = nc.dram_tensor("hidp", [D, B * FREE], bf16, kind="ExternalInput")
    # packed: cols 0:128 weT, 128:256 whT, 256:768 hid batch-0 slice (one
    # early DMA delivers everything the ramp's pe-add chunks need), 768:832
    # wvs (wv at col 799 = window col 31), 832:960 zeros (opener weights)
    wpack = nc.dram_tensor("wpack", [E, 960], bf16, kind="ExternalInput")
    batt = nc.dram_tensor("batt", [K, 1], f32, kind="ExternalInput")
    # out[b, r, g, :] holds seq s = 4r + g -> flattens to [B, S, FREE]
    out = nc.dram_tensor("out", [B, S // 4, 4, FREE], f32, kind="ExternalOutput")

    with tile.TileContext(nc) as tc, ExitStack() as ctx:
        consts = ctx.enter_context(tc.tile_pool(name="consts", bufs=1))
        encp = ctx.enter_context(tc.tile_pool(name="encp", bufs=4))
        epsum = ctx.enter_context(tc.tile_pool(name="epsum", bufs=3, space="PSUM"))
        # one score bank per batch, 2 alternating so batch b+1's scores
        # start while batch b's softmax tail drains
        scpsum = ctx.enter_context(tc.tile_pool(name="scpsum", bufs=2, space="PSUM"))
        ebp = ctx.enter_context(tc.tile_pool(name="ebp", bufs=4))
        thp = ctx.enter_context(tc.tile_pool(name="thp", bufs=4))
        smax = ctx.enter_context(tc.tile_pool(name="smax", bufs=2))

        # ---- consts (enc chunk 0 is issued by the batch loop on the
        # gpsimd queue and overlaps this single early sync-queue load;
        # everything the ramp's pe-add chunks need arrives in ONE DMA so
        # the first tanh isn't gated on a second ~2us DMA-pipe landing)
        wmain_sb = consts.tile([E, 768], bf16)
        nc.sync.dma_start(out=wmain_sb, in_=wpack[:, 0:768])
        weT_sb = wmain_sb[:, 0:128]
        whT_sb = wmain_sb[:, 128:256]
        hid0_v = wmain_sb[:, 256:768]
        # batt on the scalar queue: lands before the first tanh needs it,
        # without joining the sync queue's ramp DMA herd
        batt_sb = consts.tile([K, 1], f32)
        nc.scalar.dma_start(out=batt_sb, in_=batt[:])
        hid_sb = consts.tile([D, B * FREE], bf16)
        waux_sb = consts.tile([E, 192], bf16)
        wvs_sb = waux_sb[:, 0:64]
        zeros_sb = waux_sb[:, 64:192]
        late_state = {"done": False}

        def hid_slice(b):
            return hid0_v if b == 0 else hid_sb[:, b * FREE : (b + 1) * FREE]

        def emit_late_consts():
            # deferred off the ramp critical path (first needed: batch 0's
            # score flush / batch 1's pe-adds)
            if late_state["done"]:
                return
            late_state["done"] = True
            nc.sync.dma_start(out=waux_sb, in_=wpack[:, 768:960])
            nc.sync.dma_start(out=hid_sb[:, FREE:], in_=hidp[:, FREE:])

        # sum-matmul mask: 1.0 at the 64 valid score rows (32g+q, q<16).
        # Emitted lazily (after batch 0's ramp) so the DVE memsets don't
        # delay the first energy adds; first use is batch 0's softmax tail
        # early in batch 1.
        msk_tmp = consts.tile([128, 128], f32)
        mask_sb = consts.tile([128, 128], f32r)
        mask_state = {"done": False}

        def emit_mask():
            if mask_state["done"]:
                return
            mask_state["done"] = True
            nc.vector.memset(msk_tmp, 0.0)
            for g in range(4):
                nc.vector.memset(msk_tmp[32 * g : 32 * g + 16, :], 1.0)
            nc.vector.tensor_copy(mask_sb, msk_tmp)

        # proj_h (repeated 2x along free) per batch, f32 in SBUF
        projh2_sb = consts.tile([K, B, 2 * FREE], f32)

        def emit_projh(b):
            ph_ps = epsum.tile([K, 2 * FREE], f32, tag="e_ps", name="ph_ps")
            for jj in range(2):
                nc.tensor.matmul(ph_ps[:, jj * FREE : (jj + 1) * FREE],
                                 lhsT=whT_sb, rhs=hid_slice(b),
                                 start=True, stop=True)
            nc.vector.tensor_copy(projh2_sb[:, b, :], ph_ps)

        def softmax_tail(b, sc, expv, split=1, do_exp=True):
            # exp + sum over the 64 valid rows via masked ones-matmul,
            # broadcast to all 128 rows. The sums go to a free epsum tile
            # (one PSUM bank per split) so sum[p+1] doesn't serialize
            # behind recip[p] reading the same bank. exp lives here (off
            # the batch-boundary critical path).
            rec = smax.tile([128, FREE], f32, tag="rec", name="rec")
            ob = smax.tile([128, FREE], f32, tag="ob", name="ob")
            fs = FREE // split
            last = b == B - 1
            # last batch: sums go to a free epsum tile (one bank per split,
            # no serialization; the chunk pipeline is done so the ring is
            # idle). Mid-batch tails overwrite the consumed score bank so
            # they don't steal an e_ps ring slot from the chunk pipeline.
            if last:
                su = epsum.tile([K, 2 * FREE], f32, tag="e_ps", name="su")
                sslot = lambda p: su[:, p * FREE : p * FREE + fs]
            else:
                sslot = lambda p: sc[:, p * fs : p * fs + fs]
            # out DMAs spread across queues so their ~0.6us issue slots
            # don't serialize (matters for the final batch's drain)
            queues = ([nc.sync, nc.gpsimd, nc.scalar, nc.sync] if last
                      else [nc.sync, nc.sync, nc.gpsimd, nc.gpsimd])
            if do_exp:
                for p in range(split):
                    sl = slice(p * fs, (p + 1) * fs)
                    nc.scalar.activation(expv[:, sl], sc[:, sl], AF.Exp)
            for p in range(split):
                sl = slice(p * fs, (p + 1) * fs)
                nc.tensor.matmul(sslot(p), lhsT=mask_sb, rhs=expv[:, sl],
                                 start=True, stop=True, skip_group_check=True)
                nc.vector.reciprocal_approx_fast(out=rec[:, sl],
                                                 in_=sslot(p))
                # mul on the (otherwise idle) gpsimd engine keeps the
                # in-order DVE queue free for the energy adds; the last
                # batch splits muls across gpsimd+vector (both idle then)
                mule = nc.vector if (last and p == 1) else nc.gpsimd
                mule.tensor_mul(out=ob[:, sl], in0=expv[:, sl],
                                in1=rec[:, sl])
                for g in range(4):
                    queues[g].dma_start(
                        out=out[b, :, g, sl],
                        in_=ob[32 * g : 32 * g + 16, sl])

        pending_tail = None
        pending_scores = []   # (th, sbase, nseq, gci_created, sc_tile)
        gstate = {"ci": 0}    # global chunk counter (score-flush aging)

        def flush_scores(min_age=0, upto_batch=None):
            # emit score matmuls whose tanh was issued >= min_age chunks
            # ago: keeps the PE's in-order queue from stalling on a
            # just-issued tanh. Entries carry their own score bank, so
            # flushing crosses batch boundaries (no boundary burst).
            while pending_scores:
                th_p, sbase, nseq, cic, sc_t, eb_t = pending_scores[0]
                if upto_batch is not None:
                    if eb_t > upto_batch:
                        break
                elif gstate["ci"] - cic < min_age:
                    break
                pending_scores.pop(0)
                for jj in range(nseq):
                    s = sbase + jj
                    r, g = s // 4, s % 4
                    nc.tensor.matmul(
                        sc_t[32 * g : 32 * g + 32, :],
                        lhsT=wvs_sb[:, 31 - r : 63 - r],
                        rhs=th_p[:, jj * FREE : (jj + 1) * FREE],
                        start=False, stop=(s == S - 1),
                        skip_group_check=True,
                        tile_position=(0, 32 * g))

        for b in range(B):
            plan = _plan(b)
            dmas = _dma_plan(b)
            # score bank: opener matmul writes zeros to all 128 rows with
            # start=True (start clears has_written for the whole bank, so
            # only one matmul may use it); scores accumulate with
            # start=False. seq s -> col group s%4 row s//4, so consecutive
            # scores hit different PE col groups and run concurrently.
            sc = scpsum.tile([128, FREE], f32, tag="sc", name="sc")

            def emit_opener(sc=sc):
                nc.tensor.matmul(sc, lhsT=zeros_sb, rhs=hid0_v,
                                 start=True, stop=False,
                                 skip_group_check=True)

            if b > 0:
                emit_opener()
            expv = smax.tile([128, FREE], f32r, tag="expv", name="expv")

            # iterators over enc DMA tiles
            dma_iter = iter(dmas)
            dma_left = 0
            et = None
            et_off = 0     # first seq index held by current et tile
            s0 = 0         # next seq index to compute
            gi = 0         # group counter (for ramp bookkeeping)

            def next_dma_tile():
                nonlocal dma_left, et, et_off
                csz, q = next(dma_iter)
                et = encp.tile([E, SCH * FREE], bf16, tag="et", name="et")
                (nc.scalar if q == 'a' else nc.gpsimd).dma_start(
                    out=et[:, : csz * FREE],
                    in_=enc[b, :, s0 * FREE : (s0 + csz) * FREE])
                et_off = s0
                dma_left = csz

            for mode, chunks in plan:
                nseq = sum(chunks)
                if mode == 'E':
                    eb = ebp.tile([K, 3 * 2 * FREE], bf16, tag="eb", name="eb")
                    th = thp.tile([K, 3 * 2 * FREE], bf16, tag="the", name="the")
                else:
                    th = thp.tile([K, 2 * FREE], bf16, tag="thp", name="thp")
                off = 0
                for csz in chunks:
                    if dma_left < csz:
                        assert dma_left == 0, (b, mode, chunks, dma_left)
                        next_dma_tile()
                    j0 = s0 - et_off
                    e_ps = epsum.tile([K, 2 * FREE], f32, tag="e_ps",
                                      name="e_ps")
                    for jj in range(csz):
                        nc.tensor.matmul(
                            e_ps[:, jj * FREE : (jj + 1) * FREE],
                            lhsT=weT_sb,
                            rhs=et[:, (j0 + jj) * FREE : (j0 + jj + 1) * FREE],
                            start=True, stop=(mode == 'E'))
                    if mode == 'P':
                        for jj in range(csz):
                            nc.tensor.matmul(
                                e_ps[:, jj * FREE : (jj + 1) * FREE],
                                lhsT=whT_sb, rhs=hid_slice(b),
                                start=False, stop=True)
                    # deep in the final batch, deferral stops paying: the
                    # PE has nothing after the last scores, so flush
                    # immediately to shorten the tanh->score->exp drain
                    age = 2
                    if b == B - 1 and s0 >= 52:
                        age = 0 if s0 >= 56 else 1
                    flush_scores(min_age=age)
                    if mode == 'E':
                        nc.vector.tensor_add(
                            out=eb[:, off * FREE : (off + csz) * FREE],
                            in0=e_ps[:, : csz * FREE],
                            in1=projh2_sb[:, b, : csz * FREE])
                    else:
                        nc.scalar.activation(th[:, : csz * FREE],
                                             e_ps[:, : csz * FREE],
                                             AF.Tanh, bias=batt_sb)
                        pending_scores.append((th, s0, csz, gstate["ci"],
                                               sc, b))
                    off += csz
                    s0 += csz
                    dma_left -= csz
                    gstate["ci"] += 1
                if mode == 'E':
                    nc.scalar.activation(th[:, : nseq * FREE],
                                         eb[:, : nseq * FREE],
                                         AF.Tanh, bias=batt_sb)
                    pending_scores.append((th, s0 - nseq, nseq,
                                           gstate["ci"] - 1, sc, b))
                gi += 1
                if b == 0 and gi == 1:
                    emit_late_consts()
                    emit_opener()
                    emit_projh(0)
                if b == 0 and gi == 4:
                    emit_mask()
                if gi == 6 and b + 1 < B:
                    # next batch's proj_h mid-batch: its PE matmuls and DVE
                    # copy land where there's slack, not at the boundary
                    emit_projh(b + 1)
                if gi == 2 and pending_tail is not None:
                    flush_scores(upto_batch=pending_tail[0])
                    softmax_tail(*pending_tail)
                    pending_tail = None
            if b < B - 1:
                pending_tail = (b, sc, expv)
            else:
                flush_scores()
                softmax_tail(b, sc, expv, split=2)
    nc.compile()
    return nc


def _get_bass():
    if "nc" not in _CACHE:
        _CACHE["nc"] = _build_bass()
    return _CACHE["nc"]


def kernel(hidden_state, encoder_outputs, W_att, b_att, W_v):
    from concourse.bass_utils import run_bass_kernel_spmd

    bf16 = ml_dtypes.bfloat16
    hidden_state = np.asarray(hidden_state, dtype=np.float32)
    W_att = np.asarray(W_att, dtype=np.float32)
    b_att = np.asarray(b_att, dtype=np.float32)
    W_v = np.asarray(W_v, dtype=np.float32)
    enc_bf = np.asarray(encoder_outputs, dtype=np.float32).astype(bf16)

    wpack0 = np.zeros((E, 960), dtype=np.float32)
    wpack0[:, 0:128] = W_att[:, D:].T         # weT
    wpack0[:, 128:256] = W_att[:, :D].T       # whT
    wpack0[:, 768 + 31] = W_v[0]              # wvs window, wv at col 31
    wpack0 = wpack0.astype(bf16)
    batt = np.ascontiguousarray(b_att.reshape(K, 1))

    in_maps = []
    for c in range(NCORES):
        h0 = c * HSH
        enc_c = np.ascontiguousarray(
            enc_bf[:, :, :, h0 : h0 + HSH, :]
        ).reshape(B, E, S * FREE)
        hid_c = np.ascontiguousarray(
            hidden_state[:, :, h0 : h0 + HSH, :].transpose(1, 0, 2, 3)
        ).reshape(D, B * FREE).astype(bf16)
        wpack = wpack0.copy()
        wpack[:, 256:768] = hid_c[:, :FREE]   # hid batch-0 slice
        in_maps.append(
            {"enc": enc_c, "hidp": hid_c, "wpack": wpack, "batt": batt}
        )

    nc = _get_bass()
    kwargs = dict(_CACHE.get("run_kwargs", {}))
    res = run_bass_kernel_spmd(nc, in_maps, core_ids=list(range(NCORES)), **kwargs)
    _CACHE["last_result"] = res
    shards = [r["out"].reshape(B, S, HSH, W) for r in res.results]
    return np.concatenate(shards, axis=2)
